# revision 12
# baseline (speedup 1.0000x reference)
"""Chamfer distance kernel for 8 Trainium2 NeuronCores.

Strategy
--------
pred/target: [B=4, 8192, 3] fp32.  Output: scalar fp32.

Sharding: core c handles batch b = c//2, half h = c%2:
  pass A: pred rows  [h*4096,(h+1)*4096) x ALL 8192 targets -> d_pt
          (complete row mins for those pred rows)
  pass B: target rows[h*4096,(h+1)*4096) x ALL 8192 preds   -> d_tp
          (complete row mins for those target rows)
Both passes have identical [4096 x 8192] shape; host combine is a pure
mean (every min value is complete on exactly one core).

Distances via the GEMM cross-term trick evaluated ENTIRELY as bf16
matmuls with fp32-grade accuracy: each fp32 operand is split into 3
bf16 terms (8+8+8 mantissa bits >= fp32's 24) and the required
products are laid out along the contraction dimension:

  dist[n,m] = |p_n|^2 + |t_m|^2 - 2 p.t = sum_k L[k,n] * R[k,m]

K = 24 bf16 rows: per coordinate, the 6 split-product pairs whose sum
equals p*(-2t) to O(2^-26) rel; plus 3 rows for |p|^2 (vs ones) and 3
for |t|^2.  bf16 matmuls run 1 cycle/row vs native fp32's 4.

All transposes/splits happen on the host in numpy; the device kernel
is pure matmul + min-reduce.  PSUM evacuation (the throughput wall) is
split per 2048-wide PSUM group so BOTH consumer engines work in
parallel:
  - DVE tensor_reduce(min) consumes cols [0, SD) directly from PSUM
  - ACT copies cols [SD, 2048) to SBUF f16; DVE later folds those
    copies at 2x f16 tensor_tensor(min) rate, one small reduce per
    output chunk.
"""

import os
import sys

import numpy as np

if "/opt/trn_rl_repo" not in sys.path and os.path.isdir("/opt/trn_rl_repo"):
    sys.path.append("/opt/trn_rl_repo")

import ml_dtypes

import concourse.bacc as bacc
import concourse.mybir as mybir
from concourse import tile
from concourse.bass_utils import run_bass_kernel_spmd

BF16 = ml_dtypes.bfloat16
F32 = np.float32
F64 = np.float64

B = 4
N = 8192  # pred points per batch
M = 8192  # target points per batch
D = 3
CORES = 8
SHARD = N // 2  # rows per core per pass (4096)
K = 24  # contraction rows after bf16 splitting

GROUP = 2048  # PSUM group width (4 banks = half of PSUM)
MM_N = 512  # moving free dim per matmul (1 PSUM bank fp32)
BIG = 3.0e38  # "+inf" for min identity
SD = 512  # per-group cols consumed by DVE directly from PSUM


def _split3(x64):
    """Split float64 array into 3 bf16 terms summing to ~fp32 accuracy."""
    h = x64.astype(BF16)
    r = x64 - h.astype(F64)
    m = r.astype(BF16)
    r2 = r - m.astype(F64)
    l = r2.astype(BF16)
    return h, m, l


def _cross_rows(a3, b3):
    """Given 3-term splits of two coordinate arrays, return the 6 row
    pairs whose products sum to a*b with O(2^-26) relative error."""
    ah, am, al = a3
    bh, bm, bl = b3
    return [(ah, bh), (ah, bm), (am, bh), (ah, bl), (am, bm), (al, bh)]


def _panels(x_shard, y_full):
    """Operand panels for one pass: out[n_shard, m_full] distances.

    lhsT rows come from x_shard (stationary side), rhs rows from
    -2*y_full, plus |x|^2 (vs ones) and |y|^2 rows.
    """
    n = x_shard.shape[0]
    m = y_full.shape[0]
    x64 = x_shard.astype(F64)
    y64 = y_full.astype(F64)
    xn3 = _split3((x64 * x64).sum(-1))
    yn3 = _split3((y64 * y64).sum(-1))
    ones_n = np.ones(n, BF16)
    ones_m = np.ones(m, BF16)

    lhs_rows, rhs_rows = [], []
    for c in range(D):
        xs = _split3(x64[:, c])
        ys = _split3(-2.0 * y64[:, c])
        for la, ra in _cross_rows(xs, ys):
            lhs_rows.append(la)
            rhs_rows.append(ra)
    for i in range(3):
        lhs_rows.append(xn3[i])
        rhs_rows.append(ones_m)
    for i in range(3):
        lhs_rows.append(ones_n)
        rhs_rows.append(yn3[i])
    return (
        np.ascontiguousarray(np.stack(lhs_rows)),  # [K, n]
        np.ascontiguousarray(np.stack(rhs_rows)),  # [K, m]
    )


def build_in_maps(pred, target, shard=SHARD, full=M):
    pred = np.asarray(pred, F32)
    target = np.asarray(target, F32)
    in_maps = []
    for c in range(CORES):
        b, h = divmod(c, 2)
        p_sh = pred[b, h * shard : (h + 1) * shard]
        t_sh = target[b, h * shard : (h + 1) * shard]
        p_full = pred[b, :full]
        t_full = target[b, :full]
        a_lhs, a_rhs = _panels(p_sh, t_full)
        b_lhs, b_rhs = _panels(t_sh, p_full)
        in_maps.append(
            {"a_lhs": a_lhs, "a_rhs": a_rhs, "b_lhs": b_lhs, "b_rhs": b_rhs}
        )
    return in_maps


def build_nc(shard=SHARD, full=M, sd=SD, prio_off=40, sd0_mod=0):
    """Build + compile the per-core Bass program (SPMD across 8 cores)."""
    assert shard % 128 == 0 and full % GROUP == 0
    chunks = shard // 128  # out-row chunks per pass
    gpc = full // GROUP  # PSUM groups per chunk
    qg = GROUP // MM_N  # matmuls per group
    sa = GROUP - sd  # cols copied to f16 per group
    assert sa % 16 == 0 and sd % 16 == 0
    slots = gpc + 1  # acc slots per chunk (direct partials + fold final)

    nc = bacc.Bacc()
    dbf = mybir.dt.bfloat16
    df32 = mybir.dt.float32
    df16 = mybir.dt.float16
    vmin = mybir.AluOpType.min

    a_lhs_d = nc.dram_tensor("a_lhs", [K, shard], dbf, kind="ExternalInput")
    a_rhs_d = nc.dram_tensor("a_rhs", [K, full], dbf, kind="ExternalInput")
    b_lhs_d = nc.dram_tensor("b_lhs", [K, shard], dbf, kind="ExternalInput")
    b_rhs_d = nc.dram_tensor("b_rhs", [K, full], dbf, kind="ExternalInput")
    out_d = nc.dram_tensor("out", [128, 2 * chunks], df32, kind="ExternalOutput")

    with tile.TileContext(nc) as tc:
        with (
            tc.tile_pool(name="ops", bufs=1) as ops,
            tc.tile_pool(name="acc", bufs=1) as accp,
            tc.tile_pool(name="psum", bufs=2, space="PSUM") as psum,
            tc.tile_pool(name="cpool", bufs=6) as cpool,
            tc.tile_pool(name="spool", bufs=4) as spool,
            tc.tile_pool(name="fpool", bufs=4) as fpool,
        ):
            a_lhs = ops.tile([K, shard], dbf, tag="a_lhs")
            a_rhs = ops.tile([K, full], dbf, tag="a_rhs")
            b_lhs = ops.tile([K, shard], dbf, tag="b_lhs")
            b_rhs = ops.tile([K, full], dbf, tag="b_rhs")
            acc_a = accp.tile([128, chunks * slots], df32, tag="acc_a")
            acc_b = accp.tile([128, chunks * slots], df32, tag="acc_b")
            d_sb = accp.tile([128, 2 * chunks], df32, tag="d_sb")

            nc.sync.dma_start(a_lhs[:], a_lhs_d[:])
            nc.sync.dma_start(a_rhs[:], a_rhs_d[:])
            nc.sync.dma_start(b_lhs[:], b_lhs_d[:])
            nc.sync.dma_start(b_rhs[:], b_rhs_d[:])

            nc.vector.memset(acc_a[:], BIG)
            nc.vector.memset(acc_b[:], BIG)

            def fill_group(lw, rhs_sb, g):
                ps = psum.tile([128, GROUP], df32, tag="ps")
                # fill ACT's banks (1..qg-1) before DVE's bank 0 so the
                # bigger PSUM-evacuation op can start one matmul earlier
                for q in list(range(1, qg)) + [0]:
                    col = g * GROUP + q * MM_N
                    nc.tensor.matmul(
                        ps[:, q * MM_N : (q + 1) * MM_N],
                        lw,
                        rhs_sb[:, col : col + MM_N],
                        start=True,
                        stop=True,
                    )
                return ps

            def do_chunk(lhs_sb, rhs_sb, acc, ch, sd_ch):
                sa_ch = GROUP - sd_ch
                lw = lhs_sb[:, ch * 128 : (ch + 1) * 128]
                base = ch * slots
                cs = []
                for g in range(gpc):
                    ps = fill_group(lw, rhs_sb, g)
                    # The two PSUM-evacuating ops free the psum slot; give
                    # them scheduling priority over queued fold work so the
                    # slot cycle (the kernel's critical resource) stays short.
                    with tc.high_priority(offset=prio_off):
                        if sd_ch:
                            # DVE consumes [0, sd) directly from PSUM (bank 0)
                            nc.vector.tensor_reduce(
                                acc[:, base + g : base + g + 1],
                                ps[:, :sd_ch],
                                axis=mybir.AxisListType.X,
                                op=vmin,
                            )
                        # ACT evacuates [sd, GROUP) to f16
                        c = cpool.tile([128, sa_ch], df16, tag="cp")
                        nc.scalar.copy(c[:], ps[:, sd_ch:])
                    cs.append(c)
                # fold the f16 copies pairwise at 2x rate
                while len(cs) > 1:
                    nxt = []
                    for i in range(0, len(cs) - 1, 2):
                        mm = spool.tile([128, sa_ch], df16, tag="m")
                        nc.vector.tensor_tensor(
                            mm[:], cs[i][:], cs[i + 1][:], op=vmin
                        )
                        nxt.append(mm)
                    if len(cs) % 2:
                        nxt.append(cs[-1])
                    cs = nxt
                cur = cs[0]
                sz = sa_ch
                while sz > 160 and sz % 2 == 0:
                    sz //= 2
                    ft = fpool.tile([128, sz], df16, tag="ft")
                    nc.vector.tensor_tensor(
                        ft[:], cur[:, :sz], cur[:, sz : 2 * sz], op=vmin
                    )
                    cur = ft
                nc.vector.tensor_reduce(
                    acc[:, base + gpc : base + gpc + 1],
                    cur[:],
                    axis=mybir.AxisListType.X,
                    op=vmin,
                )

            # interleave the two passes' chunks so the scheduler always has
            # independent ready work to fill dependency stalls.  A fraction
            # of chunks run sd=0 (pure ACT copy, no bank conflict) so the
            # DVE/ACT average split lands between the bank-aligned points.
            idx = 0
            for ch in range(chunks):
                for args in ((a_lhs, a_rhs, acc_a), (b_lhs, b_rhs, acc_b)):
                    sd_ch = 0 if sd0_mod and (idx % sd0_mod) == (sd0_mod - 1) else sd
                    do_chunk(*args, ch, sd_ch)
                    idx += 1

            nc.vector.tensor_reduce(
                d_sb[:, 0:chunks],
                acc_a[:].rearrange("p (c s) -> p c s", s=slots),
                axis=mybir.AxisListType.X,
                op=vmin,
            )
            nc.vector.tensor_reduce(
                d_sb[:, chunks : 2 * chunks],
                acc_b[:].rearrange("p (c s) -> p c s", s=slots),
                axis=mybir.AxisListType.X,
                op=vmin,
            )
            nc.sync.dma_start(out_d[:], d_sb[:])

    nc.compile()
    return nc


def combine(outs, shard=SHARD, full=M):
    """outs = list of 8 [128, 2*chunks] arrays -> scalar chamfer value.

    Every min (pred-row mins in cols [0,chunks), target-row mins in
    cols [chunks,2*chunks)) is complete on exactly one core, so the
    result is just the mean of each half over all cores.
    """
    chunks = shard // 128
    a = np.stack([o[:, :chunks] for o in outs]).astype(F64)
    b = np.stack([o[:, chunks:] for o in outs]).astype(F64)
    return np.float32(a.mean() + b.mean())


_NC_CACHE = {}


def kernel(pred, target):
    key = (SHARD, M, SD)
    if key not in _NC_CACHE:
        _NC_CACHE[key] = build_nc()
    nc = _NC_CACHE[key]
    in_maps = build_in_maps(pred, target)
    res = run_bass_kernel_spmd(nc, in_maps, core_ids=list(range(CORES)))
    outs = [res.results[c]["out"] for c in range(CORES)]
    return combine(outs)
